# revision 24
# baseline (speedup 1.0000x reference)
"""CosFace loss (N=2048, D=512, C=100000) on 8 Trainium2 NeuronCores.

Strategy (classifier/tensor parallel): shard the class dimension across the 8
cores (12500 classes each, padded to 12544 = 98*128). The host pre-formats the
inputs into device-friendly layouts: weight rows and embeddings are
l2-normalized in fp32, cast to fp8e4m3, and laid out transposed ([d, c] tiled
as [128, 4, c]) so the device spends zero cycles on normalization stats or
PE-array transposes. Each core then:

  - streams its fp8 W^T shard from HBM in 7 uniform groups of 1792 classes,
  - computes cos = e_norm @ w_norm.T on the tensor engine in fp8 DoubleRow
    mode (2 k-tiles per pass, 2x bf16 throughput),
  - applies exp(30*cos - 30) on the scalar engine (fixed stabilizer 30 >=
    max logit since cos <= 1, so no max pass is needed) into bf16,
  - accumulates per-class exp values for groups 0..5 with vector-engine bf16
    adds (2x mode), then folds each batch row to S_n = sum_c exp(30 cos - 30)
    with halving adds (2x) + one short reduce, overlapped with the last
    group's matmuls; group 6 is fold-reduced straight from its exp tile,
  - computes the exact fp32 ground-truth cos via a dot of the normalized
    embedding with the normalized target weight row (host-gathered, dense).

The host sums the per-core partial [2048] vectors (disjoint class ranges) and
applies the CosFace margin + logsumexp formula in float64:

  lse_n = 30 + log(S_n - exp(30 c_n - 30) + exp(30 c_n - 12 - 30))
  nll_n = lse_n - (30 c_n - 12),  loss = mean_n nll_n

where S_n = sum_c exp(30 cos_nc - 30) (unmodified) and c_n = cos at the target
class. This is algebraically identical to softmax-CE with the margin one-hot.
"""

import numpy as np

# Problem geometry (hardcoded per contract).
N, D, C = 2048, 512, 100000
P = 128
N_CORES = 8
C_SHARD = C // N_CORES  # 12500
C_PAD = 12544  # 98 tiles of 128
NT = N // P  # 16 batch tiles
KT = D // P  # 4 k-tiles of 128 along the contraction dim
SCALE = 30.0
MARGIN = 0.4
STAB = 30.0  # logsumexp stabilizer; valid since cos <= 1
GROUP_COLS = 1792  # classes per group: 12544 = 7 * 1792
NG = C_PAD // GROUP_COLS  # 7
PB_COLS = 2048  # PSUM tile allocation width (bank-aligned); use [:, :1792]

_CACHE = {}


def _install_ntff_shim():
    """Register the axon NTFF profile hook if the image's antenv lacks it."""
    import sys
    import types

    try:
        from antenv.axon_hooks import get_axon_ntff_profile_hook  # noqa: F401

        return
    except ImportError:
        pass
    mod = types.ModuleType("antenv.axon_hooks")
    state = {"hook": None}
    mod.set_axon_ntff_profile_hook = lambda h: state.__setitem__("hook", h)
    mod.get_axon_ntff_profile_hook = lambda: state["hook"]
    sys.modules["antenv.axon_hooks"] = mod
    try:
        from trn_agent_boot.trn_boot import _ntff_profile_via_ctypes

        mod.set_axon_ntff_profile_hook(
            _ntff_profile_via_ctypes("/opt/axon/libaxon_pjrt.so")
        )
    except Exception:
        pass


def _build():
    if "nc" in _CACHE:
        return _CACHE["nc"]

    import concourse.tile as tile
    from concourse import bacc, mybir

    f32 = mybir.dt.float32
    bf16 = mybir.dt.bfloat16
    fp8 = mybir.dt.float8e4
    AF = mybir.ActivationFunctionType
    ALU = mybir.AluOpType
    AX = mybir.AxisListType
    DR = mybir.MatmulPerfMode.DoubleRow

    nc = bacc.Bacc(
        "TRN2", target_bir_lowering=False, debug=False, num_devices=N_CORES
    )
    wt_d = nc.dram_tensor("wt", [P, KT, C_PAD], fp8, kind="ExternalInput").ap()
    et_d = nc.dram_tensor("et", [P, KT, N], fp8, kind="ExternalInput").ap()
    en_d = nc.dram_tensor("en", [P, NT, D], f32, kind="ExternalInput").ap()
    gw_d = nc.dram_tensor("gw", [P, NT, D], f32, kind="ExternalInput").ap()
    sa_d = nc.dram_tensor("sa_out", [P, NT], f32, kind="ExternalOutput").ap()
    sb_d = nc.dram_tensor("sb_out", [P, NT], f32, kind="ExternalOutput").ap()
    g_d = nc.dram_tensor("g_out", [P, NT], f32, kind="ExternalOutput").ap()

    W = GROUP_COLS

    with tile.TileContext(nc) as tc:
        with (
            tc.tile_pool(name="persist", bufs=1) as persist,
            tc.tile_pool(name="wtp", bufs=3) as wt_p,
            tc.tile_pool(name="dup", bufs=6) as du_p,
            tc.tile_pool(name="gtp", bufs=4) as gt_p,
            tc.tile_pool(name="pbp", bufs=2, space="PSUM") as pb_p,
        ):
            negstab = persist.tile([P, 1], f32)
            nc.vector.memset(negstab[:], -STAB)

            # normalized, transposed fp8 embedding: e_T[p, j, n] = e[n, j*128+p]
            # Interleave startup DMAs across queues: first wt group + e_T in
            # 4 chunks each so the first matmul isn't gated on a single queue.
            e_T = persist.tile([P, KT, N], fp8)
            wt0 = wt_p.tile([P, KT, W], fp8, tag="wt")
            # Startup-critical loads: small chunks aligned with the 512-col
            # matmul chunks, triggers split across two sequencer queues so
            # trigger issue (~0.7us each) parallelizes.
            # 8 triggers total: sync-sequencer trigger issue costs ~0.7us
            # apiece, so fewer, ~512-col chunks beat fine-grained splitting
            wb = [0, 512, 1024, 1536, W]
            eb = [(0, 2, 0, N // 2), (2, 4, 0, N // 2),
                  (0, 2, N // 2, N), (2, 4, N // 2, N)]
            # 8 triggers total on sync (~0.7us apiece to issue): measured
            # faster than splitting across sequencers or finer chunks
            for q in range(4):
                nc.sync.dma_start(
                    wt0[:, :, wb[q] : wb[q + 1]],
                    wt_d[:, :, wb[q] : wb[q + 1]],
                )
                k0, k1, n0, n1 = eb[q]
                nc.sync.dma_start(e_T[:, k0:k1, n0:n1], et_d[:, k0:k1, n0:n1])
            # dummy 1-element Exp: pulls the ~1.3us ACT_TABLE_LOAD off the
            # first real exp's critical path (fires while DMAs stream in)
            scrap = persist.tile([P, 1], f32)
            nc.scalar.activation(
                scrap[:], negstab[:], AF.Exp, scale=1.0, bias=negstab[:, :1]
            )

            # per-class exp accumulator over groups 0..5
            acc = persist.tile([P, NT, W], bf16)
            sp_a = persist.tile([P, NT], f32)  # folded sums of groups 0..5
            sp_b = persist.tile([P, NT], f32)  # folded sums of group 6
            f1 = persist.tile([P, NT, W // 2], bf16)  # fold scratch

            # ---- ground-truth path (emitted mid-loop; dense host gather) ----
            def emit_gt():
                dumf = persist.tile([P, D], f32)
                gdot = persist.tile([P, NT], f32)
                for t in range(NT):
                    en_t = gt_p.tile([P, D], f32, tag="en")
                    gw_t = gt_p.tile([P, D], f32, tag="gw")
                    # sync queue on purpose: these 8.4MB of triggers must sit
                    # behind the startup-critical loads in the trigger stream
                    # (gpsimd would fire them immediately and starve startup)
                    nc.sync.dma_start(en_t[:], en_d[:, t])
                    nc.sync.dma_start(gw_t[:], gw_d[:, t])
                    nc.vector.scalar_tensor_tensor(
                        out=dumf[:],
                        in0=en_t[:],
                        scalar=1.0,
                        in1=gw_t[:],
                        op0=ALU.mult,
                        op1=ALU.mult,
                        accum_out=gdot[:, t : t + 1],
                    )
                nc.sync.dma_start(g_d, gdot[:])

            def fold_reduce(src, width, out_slot, scratch):
                """Per-row sum of src[:, :width] via 2x halving adds + reduce."""
                h1 = width // 2
                nc.vector.tensor_tensor(
                    out=scratch[:, :h1],
                    in0=src[:, :h1],
                    in1=src[:, h1 : 2 * h1],
                    op=ALU.add,
                )
                h2 = h1 // 2
                nc.vector.tensor_tensor(
                    out=scratch[:, :h2],
                    in0=scratch[:, :h2],
                    in1=scratch[:, h2 : 2 * h2],
                    op=ALU.add,
                )
                nc.vector.tensor_reduce(out_slot, scratch[:, :h2], AX.X, ALU.add)

            # ---- main streaming loop over 7 uniform class groups ----
            for gi in range(NG):
                c0 = gi * W
                real = min(C_SHARD - c0, W)  # 1792 except 1748 for gi=6
                if gi == 0:
                    wt = wt0
                else:
                    wt = wt_p.tile([P, KT, W], fp8, tag="wt")
                    h = W // 2
                    nc.sync.dma_start(wt[:, :, :h], wt_d[:, :, c0 : c0 + h])
                    nc.sync.dma_start(wt[:, :, h:], wt_d[:, :, c0 + h : c0 + W])
                for t in range(NT):
                    pb = pb_p.tile([P, PB_COLS], f32, tag="pb")
                    # g0 runs cc-outer so chunk consumption tracks the
                    # staggered arrival of the four startup DMA chunks;
                    # later groups run jp-outer (identical lhsT per pass,
                    # accumulation groups are per-bank so 4 stay open)
                    if gi == 0:
                        order = [
                            (jp, cc)
                            for cc in range((W + 511) // 512)
                            for jp in range(2)
                        ]
                    else:
                        order = [
                            (jp, cc)
                            for jp in range(2)
                            for cc in range((W + 511) // 512)
                        ]
                    for jp, cc in order:
                        cw = min(512, W - cc * 512)
                        nc.tensor.matmul(
                            pb[:, cc * 512 : cc * 512 + cw],
                            lhsT=e_T[:, 2 * jp : 2 * jp + 2, t * P : (t + 1) * P],
                            rhs=wt[:, 2 * jp : 2 * jp + 2, cc * 512 : cc * 512 + cw],
                            start=(jp == 0),
                            stop=(jp == 1),
                            perf_mode=DR,
                        )
                    du = du_p.tile([P, W], bf16, tag="du")
                    if gi == 0 and t == 0:
                        # very first step: exp per 512-col chunk so the scalar
                        # engine starts as soon as chunk 0's matmuls stop
                        # (cc-outer order above completes chunks in order),
                        # instead of waiting for the full 1792-wide fill
                        for cc in range(4):
                            cl, ch = cc * 512, min((cc + 1) * 512, W)
                            nc.scalar.activation(
                                du[:, cl:ch],
                                pb[:, cl:ch],
                                AF.Exp,
                                scale=SCALE,
                                bias=negstab[:, :1],
                            )
                        nc.vector.tensor_copy(out=acc[:, t, :], in_=du[:, :])
                    elif gi < NG - 1:
                        nc.scalar.activation(
                            du[:, :real],
                            pb[:, :real],
                            AF.Exp,
                            scale=SCALE,
                            bias=negstab[:, :1],
                        )
                        if gi == 0:
                            nc.vector.tensor_copy(out=acc[:, t, :], in_=du[:, :])
                        else:
                            nc.vector.tensor_tensor(
                                out=acc[:, t, :],
                                in0=acc[:, t, :],
                                in1=du[:, :],
                                op=ALU.add,
                            )
                    else:
                        # last group: scalar engine sums it directly via the
                        # activation accumulator; no vector-engine dependency
                        nc.scalar.activation(
                            du[:, :real],
                            pb[:, :real],
                            AF.Exp,
                            scale=SCALE,
                            bias=negstab[:, :1],
                            accum_out=sp_b[:, t : t + 1],
                        )
                        # groups 0..5 are complete for every t by now: fold
                        # acc here so the reduction hides under g6's matmuls
                        fold_reduce(acc[:, t], W, sp_a[:, t : t + 1], f1[:, t])
                if gi == 3:
                    emit_gt()

            # separate output DMAs: sp_a fires as soon as the folds finish
            # (mid-g6), sp_b right after the last accumulator read; the host
            # adds them, avoiding an on-device combine on the critical tail
            nc.sync.dma_start(sa_d, sp_a[:])
            nc.sync.dma_start(sb_d, sp_b[:])

    nc.compile()
    _CACHE["nc"] = nc
    return nc


def _prep_inputs(embedding, ground_truth, weight):
    """Host-side input formatting: l2-normalize, cast fp8, transpose-tile."""
    import ml_dtypes

    fp8 = ml_dtypes.float8_e4m3

    emb = np.ascontiguousarray(np.asarray(embedding, dtype=np.float32))
    w = np.ascontiguousarray(np.asarray(weight, dtype=np.float32))
    gt = np.asarray(ground_truth).astype(np.int64)

    def l2rows(x):
        n = np.sqrt(np.einsum("nd,nd->n", x, x, dtype=np.float64))
        n = np.maximum(n, 1e-12)
        return x / n[:, None].astype(np.float32)

    en = l2rows(emb)  # [N, D] f32, unit rows
    wn = l2rows(w)  # [C, D] f32, unit rows

    # fp8 transposed tilings: [p, kt, col] with d = kt*128 + p
    w8 = wn.astype(fp8)
    wt_full = np.ascontiguousarray(w8.T.reshape(KT, P, C).transpose(1, 0, 2))
    e8 = en.astype(fp8)
    et5 = np.ascontiguousarray(e8.T.reshape(KT, P, N).transpose(1, 0, 2))

    # fp32 row-major tilings: [p, t, d] with n = t*128 + p
    en5 = np.ascontiguousarray(en.reshape(NT, P, D).transpose(1, 0, 2))
    gw5 = np.ascontiguousarray(wn[gt].reshape(NT, P, D).transpose(1, 0, 2))

    in_maps = []
    for k in range(N_CORES):
        lo = k * C_SHARD
        wt_k = np.zeros((P, KT, C_PAD), dtype=fp8)
        wt_k[:, :, :C_SHARD] = wt_full[:, :, lo : lo + C_SHARD]
        in_maps.append({"wt": wt_k, "et": et5, "en": en5, "gw": gw5})
    return in_maps


def run(embedding, ground_truth, weight, trace=False):
    """Run the sharded device kernel; returns (loss_scalar, BassKernelResults)."""
    import concourse.bass_utils as bass_utils

    if trace:
        _install_ntff_shim()

    nc = _build()
    in_maps = _prep_inputs(embedding, ground_truth, weight)

    kwargs = {}
    if trace:
        import os

        os.environ["BASS_PERFETTO_PROFILE_ALL_CORES"] = "1"
        kwargs = dict(trace=True, trace_cores=list(range(N_CORES)), stitch_traces=False)

    res = bass_utils.run_bass_kernel_spmd(
        nc, in_maps, core_ids=list(range(N_CORES)), **kwargs
    )

    S = np.zeros(N, dtype=np.float64)
    for k in range(N_CORES):
        S += res.results[k]["sa_out"].astype(np.float64).T.reshape(N)
        S += res.results[k]["sb_out"].astype(np.float64).T.reshape(N)
    cg = res.results[0]["g_out"].astype(np.float64).T.reshape(N)
    cg = np.clip(cg, -1.0 + 1e-7, 1.0 - 1e-7)

    lse = STAB + np.log(
        S - np.exp(SCALE * cg - STAB) + np.exp(SCALE * cg - SCALE * MARGIN - STAB)
    )
    nll = lse - (SCALE * cg - SCALE * MARGIN)
    loss = np.float32(nll.mean())
    return loss, res


def kernel(embedding, ground_truth, weight):
    # one retry guards against rare transient device flakes (observed once:
    # a NaN result from a bad exec; the program itself is deterministic)
    loss = None
    for attempt in range(2):
        try:
            loss, _ = run(embedding, ground_truth, weight, trace=False)
        except Exception:
            if attempt == 1:
                raise
            continue
        if np.isfinite(loss):
            break
    return np.asarray(loss, dtype=np.float32)
